# revision 13
# baseline (speedup 1.0000x reference)
"""Trainium2 Bass kernel for nn_CWDiscriminator (per-class 3-layer MLP).

reference:
    x = inputs.transpose(0, 2, 1)            # (B, C, F)
    h = relu(einsum('bcf,cfg->bcg', x, W1) + b1)
    h = relu(einsum('bcf,cfg->bcg', h, W2) + b2)
    out = einsum('bcf,cf->bc', h, W3) + b3   # (B, C)

B=16384, F=256, C=19. Data-parallel over B across 8 NeuronCores
(B_loc = 2048 per core). CLASS-MAJOR schedule: outer loop over classes,
inner over four 512-column batch sections, so HBM demand is uniform
(~1.3 MB per 8.6 us of compute) instead of front-loaded; X prefetches
several classes ahead on the sync ring while weights stream
class-by-class on the scalar ring.

Per class c, per section h:
  - GEMM1 (bf16): H1.T = W1[c].T @ X.T -> PSUM, ACT evicts fused
    bias+ReLU to bf16.
  - GEMM2 (bf16): H2.T = W2[c].T @ H1.T -> PSUM, DVE evicts fused
    bias+ReLU to bf16.
  - GEMM3: col-tiled, batched per group of 4 classes (r = c//4,
    strip j = c%4). The four classes' matmuls go to the four
    32-partition column groups via tile_position=(0,32j) and run
    concurrently, all accumulating into ONE shared PSUM bank at
    partition 32j + 5h + r (5h+r is a bijection onto 0..19, so each
    strip's 20 rows are dense); section h is selected by leading-zero
    columns in the lhsT. A zero matmul at kernel start sets
    has_written for the bank (doubling as PE warm-up), so strip
    matmuls never need start=True (whose bank-wide clear would race
    between concurrent strips).
Output per core: one padded DMA to dram [4,32,SEC-cols-per-...]; host
takes rows :20 per strip, reshapes, transposes, adds b3.
"""

import sys
import types

import numpy as np
import ml_dtypes

B, F, C = 16384, 256, 19
NCORES = 8
B_LOC = B // NCORES          # 2048
NSEC = 4
SEC = B_LOC // NSEC          # 512
BF16 = ml_dtypes.bfloat16

XT_BUFS = 7                  # X prefetch depth in classes (1 MB each)
# strip j = c % 4, row r = c // 4; classes per strip column
STRIP_N = [5, 5, 5, 4]


# ---------------------------------------------------------------------------
# axon environment shims (NTFF profile hook + artifact upload stub) and the
# one-wait-per-instruction legalizer this walrus build requires.
# ---------------------------------------------------------------------------

def _setup_axon_env():
    if 'antenv.axon_hooks' not in sys.modules:
        mod = types.ModuleType('antenv.axon_hooks')
        mod._hook = None
        mod.set_axon_ntff_profile_hook = lambda h: setattr(mod, '_hook', h)
        mod.get_axon_ntff_profile_hook = lambda: mod._hook
        sys.modules['antenv.axon_hooks'] = mod
        try:
            import antenv
            antenv.axon_hooks = mod
        except ImportError:
            pass
        try:
            from trn_agent_boot.trn_boot import _ntff_profile_via_ctypes
            mod._hook = _ntff_profile_via_ctypes('/opt/axon/libaxon_pjrt.so')
        except Exception:
            pass
    import concourse.bass_utils as bu
    bu.upload_artifacts = lambda tmpdir: 'file://' + str(tmpdir)


def _legalize_waits(nc):
    """walrus accepts at most ONE sync wait per engine instruction (2 for
    EventSemaphore). Split extras onto preceding same-engine NoOps."""
    import concourse.mybir as mybir
    n_split = 0
    for fn in nc.m.functions:
        for bb in fn.blocks:
            insts = bb.instructions
            out = []
            for inst in insts:
                si = inst.sync_info
                ow = list(si.on_wait) if si is not None and si.on_wait else []
                cap = 2 if inst.opcode == "EventSemaphore" else 1
                if len(ow) > cap:
                    keep = ow[-cap:]
                    for k, w in enumerate(ow[:-cap]):
                        nop = mybir.InstNoOp(
                            name=f"{inst.name}-wsplit{k}",
                            engine=inst.engine,
                            ins=[],
                            outs=[],
                            sync_info=mybir.SyncInfo(on_wait=[w], on_update=[]),
                        )
                        out.append(nop)
                        n_split += 1
                    inst.sync_info = mybir.SyncInfo(
                        on_wait=keep,
                        on_update=list(si.on_update) if si.on_update else [],
                    )
                out.append(inst)
            insts[:] = out
    return n_split


# ---------------------------------------------------------------------------
# device program
# ---------------------------------------------------------------------------

_CACHE = {}
last_results = None  # BassKernelResults of the most recent run (for test.py)

# class-range chunks for weight loads (finer first so class 0 starts early)
W_CHUNKS = [(0, 1), (1, 2), (2, 4), (4, 7), (7, 11), (11, 15), (15, C)]


def _build_program():
    from contextlib import ExitStack
    import concourse.bass as bass
    import concourse.mybir as mybir
    import concourse.tile as tile

    F32 = mybir.dt.float32
    B16 = mybir.dt.bfloat16

    nc = bass.Bass()

    # xtc[c, p, k, b] = x[b, 128k+p, c]  (class-major, contiguous per class)
    xtc = nc.declare_dram_parameter("xtc", [C, 128, 2, B_LOC], B16,
                                    isOutput=False)
    w1t = nc.declare_dram_parameter("w1t", [128, C, 2, 2, 128], B16,
                                    isOutput=False)
    w2t = nc.declare_dram_parameter("w2t", [128, C, 2, 2, 128], B16,
                                    isOutput=False)
    # w3z[p, c, k, h, i] = (i == 8h + c//4) * W3[c, 128k+p]
    w3z = nc.declare_dram_parameter("w3z", [128, C, 2, NSEC, 20], B16,
                                    isOutput=False)
    b1s = nc.declare_dram_parameter("b1s", [128, C, 2], F32, isOutput=False)
    b2s = nc.declare_dram_parameter("b2s", [128, C, 2], F32, isOutput=False)
    # out[j, q, b]: q = 5h+r -> class 4r+j, batch column 512h+b
    # (rows 20..31 per strip are padding so one 128-partition DMA works)
    out = nc.declare_dram_parameter("out", [4, 32, SEC], F32,
                                    isOutput=True)

    with ExitStack() as ctx:
        tc = ctx.enter_context(tile.TileContext(nc))

        consts = ctx.enter_context(tc.tile_pool(name="consts", bufs=1))
        xt_pool = ctx.enter_context(tc.tile_pool(name="xt", bufs=XT_BUFS))
        h1_pool = ctx.enter_context(tc.tile_pool(name="h1p", bufs=8))
        h2_pool = ctx.enter_context(tc.tile_pool(name="h2p", bufs=20))
        out_pool = ctx.enter_context(tc.tile_pool(name="outp", bufs=1))

        ps_g = ctx.enter_context(
            tc.tile_pool(name="ps_g", bufs=7, space="PSUM"))
        ps_3 = ctx.enter_context(
            tc.tile_pool(name="ps_3", bufs=1, space="PSUM"))

        # ---- SBUF tiles for weights/biases
        w1sb = consts.tile([128, C, 2, 2, 128], B16)
        w2sb = consts.tile([128, C, 2, 2, 128], B16)
        w3sb = consts.tile([128, C, 2, NSEC, 20], B16)
        b1sb = consts.tile([128, C, 2], F32)
        b2sb = consts.tile([128, C, 2], F32)

        # ---- DMA: X classes on the sync ring (pool-paced prefetch),
        # weights on the scalar ring in consumption order.
        xts = [xt_pool.tile([128, 2, B_LOC], B16, tag="xt",
                            name=f"xt{c}") for c in range(C)]
        nc.sync.dma_start(xts[0][:, :, 0:SEC], xtc[0, :, :, 0:SEC])
        nc.sync.dma_start(w1sb[:, 0:1], w1t[:, 0:1])
        nc.sync.dma_start(xts[0][:, :, SEC:2 * SEC],
                          xtc[0, :, :, SEC:2 * SEC])
        nc.sync.dma_start(w2sb[:, 0:1], w2t[:, 0:1])
        nc.sync.dma_start(xts[0][:, :, 2 * SEC:],
                          xtc[0, :, :, 2 * SEC:])
        nc.sync.dma_start(b1sb[:], b1s[:])
        nc.sync.dma_start(b2sb[:], b2s[:])
        nc.sync.dma_start(xts[1][:], xtc[1])
        for n, (c0, c1) in enumerate(W_CHUNKS[1:]):
            nc.sync.dma_start(w1sb[:, c0:c1], w1t[:, c0:c1])
            nc.sync.dma_start(w2sb[:, c0:c1], w2t[:, c0:c1])
            if n == 2:
                nc.sync.dma_start(w3sb[:], w3z[:])
            if n + 2 < C:
                nc.sync.dma_start(xts[n + 2][:], xtc[n + 2])
        for c in range(len(W_CHUNKS) + 1, C):
            nc.sync.dma_start(xts[c][:], xtc[c])

        # ---- zero matmuls: set has_written across all 4 GEMM3 banks and
        # warm the PE while the first DMA chunks land.
        wu_l = consts.tile([128, 128], B16)
        wu_r = consts.tile([128, 512], B16)
        nc.vector.memset(wu_l[:], 0.0)
        nc.vector.memset(wu_r[:], 0.0)
        ps3 = ps_3.tile([128, SEC], mybir.dt.float32, tag="ps3")
        nc.tensor.matmul(ps3[:], wu_l[:], wu_r[:, 0:SEC],
                         start=True, stop=False, skip_group_check=True)
        wu_ps = ps_g.tile([128, 512], mybir.dt.float32, tag="pg")
        for i in range(8):
            nc.tensor.matmul(wu_ps[:], wu_l[:], wu_r[:],
                             start=True, stop=True)

        w1v = w1sb[:]
        w2v = w2sb[:]
        w3v = w3sb[:]

        h1_t = [None, None]      # per class parity: list of 4 section tiles
        h2_t = [None] * C        # h2 tiles for the current group of 4

        def burst(r):
            # GEMM3 for classes 4r..4r+3, all sections; strips concurrent.
            # Section h lands at partition offset 5h + r within strip j via
            # leading-zero columns in the lhsT (width 5h+r+1 <= 20); 5h+r
            # is a bijection onto 0..19 so the 20 partitions per strip are
            # dense and the out-DMA is contiguous.
            for h in range(NSEC):
                for k in range(2):
                    for j in range(4):
                        c = 4 * r + j
                        if c >= C:
                            continue
                        w = 5 * h + r + 1
                        last = (r == 4 and h == NSEC - 1 and k == 1
                                and j == 2)
                        nc.tensor.matmul(
                            ps3[32 * j:32 * j + w, :],
                            w3v[:, c, k, h, 0:w],
                            h2_t[c][h][:, k, :],
                            start=False, stop=last,
                            tile_position=(0, 32 * j),
                            skip_group_check=True)

        for cc in range(C + 1):
            if cc < C:
                c = cc
                h1s = []
                for h in range(NSEC):
                    h1 = h1_pool.tile([128, 2, SEC], B16, tag="h1")
                    h1s.append(h1)
                    for m in range(2):
                        pg = ps_g.tile([128, SEC], mybir.dt.float32,
                                       tag="pg")
                        for k in range(2):
                            nc.tensor.matmul(
                                pg[:], w1v[:, c, k, m, :],
                                xts[c][:, k, h * SEC:(h + 1) * SEC],
                                start=(k == 0), stop=(k == 1))
                        nc.scalar.activation(
                            h1[:, m, :], pg[:],
                            mybir.ActivationFunctionType.Relu,
                            bias=b1sb[:, c, m:m+1])
                h1_t[c % 2] = h1s
            if cc >= 1:
                c = cc - 1
                h1s = h1_t[c % 2]
                h2s = []
                for h in range(NSEC):
                    h2 = h2_pool.tile([128, 2, SEC], B16, tag="h2")
                    h2s.append(h2)
                    for m in range(2):
                        pg = ps_g.tile([128, SEC], mybir.dt.float32,
                                       tag="pg")
                        for k in range(2):
                            nc.tensor.matmul(
                                pg[:], w2v[:, c, k, m, :],
                                h1s[h][:, k, :],
                                start=(k == 0), stop=(k == 1))
                        nc.vector.tensor_scalar(
                            h2[:, m, :], pg[:],
                            b2sb[:, c, m:m+1], 0.0,
                            mybir.AluOpType.add, mybir.AluOpType.max)
                h2_t[c] = h2s
            if cc >= 4 and cc % 4 == 0:
                burst(cc // 4 - 1)
            if cc == C:
                burst(4)

        # ---- eviction: PSUM->SBUF copy split over DVE/ACT, then one
        # 128-partition DMA (pad rows land in the dram padding).
        out_sb = out_pool.tile([128, SEC], F32, tag="osb")
        nc.vector.tensor_copy(out_sb[:, 0:SEC // 2], ps3[:, 0:SEC // 2])
        nc.scalar.activation(out_sb[:, SEC // 2:], ps3[:, SEC // 2:],
                             mybir.ActivationFunctionType.Copy)
        nc.sync.dma_start(out[:].rearrange("j q b -> (j q) b"), out_sb[:])

    _legalize_waits(nc)
    return nc


def _get_program():
    if 'nc' not in _CACHE:
        _setup_axon_env()
        _CACHE['nc'] = _build_program()
    return _CACHE['nc']


# ---------------------------------------------------------------------------
# host wrapper
# ---------------------------------------------------------------------------

def kernel(inputs, W1, b1, W2, b2, W3, b3):
    global last_results
    from concourse.bass_utils import run_bass_kernel_spmd

    nc = _get_program()

    inputs = np.asarray(inputs)
    W1 = np.asarray(W1, dtype=np.float32)
    b1 = np.asarray(b1, dtype=np.float32)
    W2 = np.asarray(W2, dtype=np.float32)
    b2 = np.asarray(b2, dtype=np.float32)
    W3 = np.asarray(W3, dtype=np.float32)
    b3 = np.asarray(b3, dtype=np.float32)

    # host-side layout prep: xtc[c, p, k, b] = x[b, 128k+p, c]
    xbf = inputs.reshape(B, 2, 128, C).astype(BF16)
    xtc_full = xbf.transpose(3, 2, 1, 0)      # (C, 128, 2, B)

    # lhsT tiles: w{1,2}t[p, c, k, m, j] = W[c, 128k+p, 128m+j]
    w1t = np.ascontiguousarray(
        W1.reshape(C, 2, 128, 2, 128).transpose(2, 0, 1, 3, 4)).astype(BF16)
    w2t = np.ascontiguousarray(
        W2.reshape(C, 2, 128, 2, 128).transpose(2, 0, 1, 3, 4)).astype(BF16)
    # w3z[p, c, k, h, i] = (i == 5h + c//4) * W3[c, 128k+p]
    w3z = np.zeros((128, C, 2, NSEC, 20), dtype=np.float32)
    for c in range(C):
        for h in range(NSEC):
            w3z[:, c, 0, h, 5 * h + c // 4] = W3[c, :128]
            w3z[:, c, 1, h, 5 * h + c // 4] = W3[c, 128:]
    w3z = w3z.astype(BF16)
    # b1s[p, c, m] = b1[c, 128m+p]
    b1s = np.ascontiguousarray(
        b1.reshape(C, 2, 128).transpose(2, 0, 1)).astype(np.float32)
    b2s = np.ascontiguousarray(
        b2.reshape(C, 2, 128).transpose(2, 0, 1)).astype(np.float32)

    core_ids = list(range(NCORES))
    in_maps = []
    for i in core_ids:
        xc = np.ascontiguousarray(
            xtc_full[:, :, :, i * B_LOC:(i + 1) * B_LOC])
        in_maps.append({
            "xtc": xc,
            "w1t": w1t, "w2t": w2t, "w3z": w3z, "b1s": b1s, "b2s": b2s,
        })

    import os
    trace = bool(os.environ.get("BASS_TRACE"))
    res = run_bass_kernel_spmd(nc, in_maps, core_ids, trace=trace)
    last_results = res

    out_full = np.empty((B, C), dtype=np.float32)
    for i in core_ids:
        o = res.results[i]["out"].reshape(4, 32, SEC)[:, :20]
        o = o.reshape(4, NSEC, 5, SEC)
        # o[j, h, r, b] = out[class 4r+j, batch 512h+b]
        o = o.transpose(1, 3, 2, 0).reshape(B_LOC, 20)[:, :C]
        out_full[i * B_LOC:(i + 1) * B_LOC] = o
    out_full += b3[None, :]
    return out_full


# revision 16
# speedup vs baseline: 1.1921x; 1.1921x over previous
"""Trainium2 Bass kernel for nn_CWDiscriminator (per-class 3-layer MLP).

reference:
    x = inputs.transpose(0, 2, 1)            # (B, C, F)
    h = relu(einsum('bcf,cfg->bcg', x, W1) + b1)
    h = relu(einsum('bcf,cfg->bcg', h, W2) + b2)
    out = einsum('bcf,cf->bc', h, W3) + b3   # (B, C)

B=16384, F=256, C=19. Data-parallel over B across 8 NeuronCores
(B_loc = 2048 per core). CLASS-MAJOR schedule: outer loop over classes,
inner over four 512-column batch sections, so HBM demand is uniform
(~1.3 MB per 8.6 us of compute) instead of front-loaded; X prefetches
several classes ahead on the sync ring while weights stream
class-by-class on the scalar ring.

Per class c, per section h:
  - GEMM1 (bf16): H1.T = W1[c].T @ X.T -> PSUM, ACT evicts fused
    bias+ReLU to bf16.
  - GEMM2 (bf16): H2.T = W2[c].T @ H1.T -> PSUM, DVE evicts fused
    bias+ReLU to bf16.
  - GEMM3: col-tiled, batched per group of 4 classes (r = c//4,
    strip j = c%4). The four classes' matmuls go to the four
    32-partition column groups via tile_position=(0,32j) and run
    concurrently, all accumulating into ONE shared PSUM bank at
    partition 32j + 5h + r (5h+r is a bijection onto 0..19, so each
    strip's 20 rows are dense); section h is selected by leading-zero
    columns in the lhsT. A zero matmul at kernel start sets
    has_written for the bank (doubling as PE warm-up), so strip
    matmuls never need start=True (whose bank-wide clear would race
    between concurrent strips).
Output per core: one padded DMA to dram [4,32,SEC-cols-per-...]; host
takes rows :20 per strip, reshapes, transposes, adds b3.
"""

import sys
import types

import numpy as np
import ml_dtypes

B, F, C = 16384, 256, 19
NCORES = 8
B_LOC = B // NCORES          # 2048
NSEC = 4
SEC = B_LOC // NSEC          # 512
BF16 = ml_dtypes.bfloat16

XT_BUFS = 8                  # X prefetch depth in classes (1 MB each)
# strip j = c % 4, row r = c // 4; classes per strip column
STRIP_N = [5, 5, 5, 4]


# ---------------------------------------------------------------------------
# axon environment shims (NTFF profile hook + artifact upload stub) and the
# one-wait-per-instruction legalizer this walrus build requires.
# ---------------------------------------------------------------------------

def _setup_axon_env():
    if 'antenv.axon_hooks' not in sys.modules:
        mod = types.ModuleType('antenv.axon_hooks')
        mod._hook = None
        mod.set_axon_ntff_profile_hook = lambda h: setattr(mod, '_hook', h)
        mod.get_axon_ntff_profile_hook = lambda: mod._hook
        sys.modules['antenv.axon_hooks'] = mod
        try:
            import antenv
            antenv.axon_hooks = mod
        except ImportError:
            pass
        try:
            from trn_agent_boot.trn_boot import _ntff_profile_via_ctypes
            mod._hook = _ntff_profile_via_ctypes('/opt/axon/libaxon_pjrt.so')
        except Exception:
            pass
    import concourse.bass_utils as bu
    bu.upload_artifacts = lambda tmpdir: 'file://' + str(tmpdir)


def _legalize_waits(nc):
    """walrus accepts at most ONE sync wait per engine instruction (2 for
    EventSemaphore). Split extras onto preceding same-engine NoOps."""
    import concourse.mybir as mybir
    n_split = 0
    for fn in nc.m.functions:
        for bb in fn.blocks:
            insts = bb.instructions
            out = []
            for inst in insts:
                si = inst.sync_info
                ow = list(si.on_wait) if si is not None and si.on_wait else []
                cap = 2 if inst.opcode == "EventSemaphore" else 1
                if len(ow) > cap:
                    keep = ow[-cap:]
                    for k, w in enumerate(ow[:-cap]):
                        nop = mybir.InstNoOp(
                            name=f"{inst.name}-wsplit{k}",
                            engine=inst.engine,
                            ins=[],
                            outs=[],
                            sync_info=mybir.SyncInfo(on_wait=[w], on_update=[]),
                        )
                        out.append(nop)
                        n_split += 1
                    inst.sync_info = mybir.SyncInfo(
                        on_wait=keep,
                        on_update=list(si.on_update) if si.on_update else [],
                    )
                out.append(inst)
            insts[:] = out
    return n_split


# ---------------------------------------------------------------------------
# device program
# ---------------------------------------------------------------------------

_CACHE = {}
last_results = None  # BassKernelResults of the most recent run (for test.py)

# class-range chunks for weight loads (finer first so class 0 starts early)
W_CHUNKS = [(0, 1), (1, 2), (2, 4), (4, 7), (7, 11), (11, 15), (15, C)]


def _build_program():
    from contextlib import ExitStack
    import concourse.bass as bass
    import concourse.mybir as mybir
    import concourse.tile as tile

    F32 = mybir.dt.float32
    B16 = mybir.dt.bfloat16

    nc = bass.Bass()

    # xtc[c, p, k, b] = x[b, 128k+p, c]  (class-major, contiguous per class)
    xtc = nc.declare_dram_parameter("xtc", [C, 128, 2, B_LOC], B16,
                                    isOutput=False)
    w1t = nc.declare_dram_parameter("w1t", [128, C, 2, 2, 128], B16,
                                    isOutput=False)
    w2t = nc.declare_dram_parameter("w2t", [128, C, 2, 2, 128], B16,
                                    isOutput=False)
    # w3z[p, c, k, h, i] = (i == 8h + c//4) * W3[c, 128k+p]
    w3z = nc.declare_dram_parameter("w3z", [128, C, 2, NSEC, 20], B16,
                                    isOutput=False)
    b1s = nc.declare_dram_parameter("b1s", [128, C, 2], F32, isOutput=False)
    b2s = nc.declare_dram_parameter("b2s", [128, C, 2], F32, isOutput=False)
    # out[j, q, b]: q = 5h+r -> class 4r+j, batch column 512h+b
    # (rows 20..31 per strip are padding so one 128-partition DMA works)
    out = nc.declare_dram_parameter("out", [4, 32, SEC], F32,
                                    isOutput=True)

    with ExitStack() as ctx:
        tc = ctx.enter_context(tile.TileContext(nc))

        consts = ctx.enter_context(tc.tile_pool(name="consts", bufs=1))
        xt_pool = ctx.enter_context(tc.tile_pool(name="xt", bufs=XT_BUFS))
        h1_pool = ctx.enter_context(tc.tile_pool(name="h1p", bufs=8))
        h2_pool = ctx.enter_context(tc.tile_pool(name="h2p", bufs=24))
        out_pool = ctx.enter_context(tc.tile_pool(name="outp", bufs=1))

        ps_g = ctx.enter_context(
            tc.tile_pool(name="ps_g", bufs=7, space="PSUM"))
        ps_3 = ctx.enter_context(
            tc.tile_pool(name="ps_3", bufs=1, space="PSUM"))

        # ---- SBUF tiles for weights/biases
        w1sb = consts.tile([128, C, 2, 2, 128], B16)
        w2sb = consts.tile([128, C, 2, 2, 128], B16)
        w3sb = consts.tile([128, C, 2, NSEC, 20], B16)
        b1sb = consts.tile([128, C, 2], F32)
        b2sb = consts.tile([128, C, 2], F32)

        # ---- DMA: X classes on the sync ring (pool-paced prefetch),
        # weights on the scalar ring in consumption order.
        xts = [xt_pool.tile([128, 2, B_LOC], B16, tag="xt",
                            name=f"xt{c}") for c in range(C)]
        nc.sync.dma_start(b1sb[:], b1s[:])
        nc.sync.dma_start(b2sb[:], b2s[:])
        nc.sync.dma_start(xts[0][:, :, 0:SEC], xtc[0, :, :, 0:SEC])
        nc.sync.dma_start(w1sb[:, 0:1], w1t[:, 0:1])
        nc.sync.dma_start(xts[0][:, :, SEC:2 * SEC],
                          xtc[0, :, :, SEC:2 * SEC])
        nc.sync.dma_start(xts[0][:, :, 2 * SEC:],
                          xtc[0, :, :, 2 * SEC:])
        nc.sync.dma_start(w2sb[:, 0:1], w2t[:, 0:1])
        nc.sync.dma_start(xts[1][:, :, 0:B_LOC // 2],
                          xtc[1, :, :, 0:B_LOC // 2])
        nc.sync.dma_start(xts[1][:, :, B_LOC // 2:],
                          xtc[1, :, :, B_LOC // 2:])
        for n, (c0, c1) in enumerate(W_CHUNKS[1:]):
            nc.sync.dma_start(w1sb[:, c0:c1], w1t[:, c0:c1])
            nc.sync.dma_start(w2sb[:, c0:c1], w2t[:, c0:c1])
            if n == 2:
                nc.sync.dma_start(w3sb[:], w3z[:])
            if n + 2 < C:
                nc.sync.dma_start(xts[n + 2][:], xtc[n + 2])
        for c in range(len(W_CHUNKS) + 1, C):
            nc.sync.dma_start(xts[c][:], xtc[c])

        # ---- zero matmuls: set has_written across all 4 GEMM3 banks and
        # warm the PE while the first DMA chunks land.
        wu_l = consts.tile([128, 128], B16)
        wu_r = consts.tile([128, 512], B16)
        nc.vector.memset(wu_l[:], 0.0)
        nc.vector.memset(wu_r[:], 0.0)
        ps3 = ps_3.tile([128, SEC], mybir.dt.float32, tag="ps3")
        nc.tensor.matmul(ps3[:], wu_l[:], wu_r[:, 0:SEC],
                         start=True, stop=False, skip_group_check=True)
        wu_ps = ps_g.tile([128, 512], mybir.dt.float32, tag="pg")
        for i in range(6):
            nc.tensor.matmul(wu_ps[:], wu_l[:], wu_r[:],
                             start=True, stop=True)

        w1v = w1sb[:]
        w2v = w2sb[:]
        w3v = w3sb[:]

        h1_t = [None, None]      # per class parity: list of 4 section tiles
        h2_t = [None] * C        # h2 tiles for the current group of 4

        def burst(r):
            # GEMM3 for classes 4r..4r+3, all sections; strips concurrent.
            # Section h lands at partition offset 5h + r within strip j via
            # leading-zero columns in the lhsT (width 5h+r+1 <= 20); 5h+r
            # is a bijection onto 0..19 so the 20 partitions per strip are
            # dense and the out-DMA is contiguous.
            for h in range(NSEC):
                for k in range(2):
                    for j in range(4):
                        c = 4 * r + j
                        if c >= C:
                            continue
                        w = 5 * h + r + 1
                        last = (r == 4 and h == NSEC - 1 and k == 1
                                and j == 2)
                        nc.tensor.matmul(
                            ps3[32 * j:32 * j + w, :],
                            w3v[:, c, k, h, 0:w],
                            h2_t[c][h][:, k, :],
                            start=False, stop=last,
                            tile_position=(0, 32 * j),
                            skip_group_check=True)

        for cc in range(C + 1):
            if cc >= 5 and (cc - 5) % 4 == 0 and cc < C:
                burst((cc - 5) // 4)
            if cc < C:
                c = cc
                h1s = []
                for h in range(NSEC):
                    h1 = h1_pool.tile([128, 2, SEC], B16, tag="h1")
                    h1s.append(h1)
                    for m in range(2):
                        pg = ps_g.tile([128, SEC], mybir.dt.float32,
                                       tag="pg")
                        for k in range(2):
                            nc.tensor.matmul(
                                pg[:], w1v[:, c, k, m, :],
                                xts[c][:, k, h * SEC:(h + 1) * SEC],
                                start=(k == 0), stop=(k == 1))
                        nc.scalar.activation(
                            h1[:, m, :], pg[:],
                            mybir.ActivationFunctionType.Relu,
                            bias=b1sb[:, c, m:m+1])
                h1_t[c % 2] = h1s
            if cc >= 1:
                c = cc - 1
                h1s = h1_t[c % 2]
                h2s = []
                for h in range(NSEC):
                    h2 = h2_pool.tile([128, 2, SEC], B16, tag="h2")
                    h2s.append(h2)
                    for m in range(2):
                        pg = ps_g.tile([128, SEC], mybir.dt.float32,
                                       tag="pg")
                        for k in range(2):
                            nc.tensor.matmul(
                                pg[:], w2v[:, c, k, m, :],
                                h1s[h][:, k, :],
                                start=(k == 0), stop=(k == 1))
                        if cc == C:
                            # last class: ACT is idle (no GEMM1 left) and
                            # evicts with no queue lag, so the final
                            # GEMM3 burst's h2 deps resolve immediately.
                            nc.scalar.activation(
                                h2[:, m, :], pg[:],
                                mybir.ActivationFunctionType.Relu,
                                bias=b2sb[:, c, m:m+1])
                        else:
                            nc.vector.tensor_scalar(
                                h2[:, m, :], pg[:],
                                b2sb[:, c, m:m+1], 0.0,
                                mybir.AluOpType.add, mybir.AluOpType.max)
                h2_t[c] = h2s
            if cc == C:
                burst(4)

        # ---- eviction: PSUM->SBUF copy split over DVE/ACT, then one
        # 128-partition DMA (pad rows land in the dram padding).
        out_sb = out_pool.tile([128, SEC], F32, tag="osb")
        nc.vector.tensor_copy(out_sb[:, 0:SEC // 2], ps3[:, 0:SEC // 2])
        nc.scalar.activation(out_sb[:, SEC // 2:], ps3[:, SEC // 2:],
                             mybir.ActivationFunctionType.Copy)
        nc.sync.dma_start(out[:].rearrange("j q b -> (j q) b"), out_sb[:])

    _legalize_waits(nc)
    return nc


def _get_program():
    if 'nc' not in _CACHE:
        _setup_axon_env()
        _CACHE['nc'] = _build_program()
    return _CACHE['nc']


# ---------------------------------------------------------------------------
# host wrapper
# ---------------------------------------------------------------------------

def kernel(inputs, W1, b1, W2, b2, W3, b3):
    global last_results
    from concourse.bass_utils import run_bass_kernel_spmd

    nc = _get_program()

    inputs = np.asarray(inputs)
    W1 = np.asarray(W1, dtype=np.float32)
    b1 = np.asarray(b1, dtype=np.float32)
    W2 = np.asarray(W2, dtype=np.float32)
    b2 = np.asarray(b2, dtype=np.float32)
    W3 = np.asarray(W3, dtype=np.float32)
    b3 = np.asarray(b3, dtype=np.float32)

    # host-side layout prep: xtc[c, p, k, b] = x[b, 128k+p, c]
    xbf = inputs.reshape(B, 2, 128, C).astype(BF16)
    xtc_full = xbf.transpose(3, 2, 1, 0)      # (C, 128, 2, B)

    # lhsT tiles: w{1,2}t[p, c, k, m, j] = W[c, 128k+p, 128m+j]
    w1t = np.ascontiguousarray(
        W1.reshape(C, 2, 128, 2, 128).transpose(2, 0, 1, 3, 4)).astype(BF16)
    w2t = np.ascontiguousarray(
        W2.reshape(C, 2, 128, 2, 128).transpose(2, 0, 1, 3, 4)).astype(BF16)
    # w3z[p, c, k, h, i] = (i == 5h + c//4) * W3[c, 128k+p]
    w3z = np.zeros((128, C, 2, NSEC, 20), dtype=np.float32)
    for c in range(C):
        for h in range(NSEC):
            w3z[:, c, 0, h, 5 * h + c // 4] = W3[c, :128]
            w3z[:, c, 1, h, 5 * h + c // 4] = W3[c, 128:]
    w3z = w3z.astype(BF16)
    # b1s[p, c, m] = b1[c, 128m+p]
    b1s = np.ascontiguousarray(
        b1.reshape(C, 2, 128).transpose(2, 0, 1)).astype(np.float32)
    b2s = np.ascontiguousarray(
        b2.reshape(C, 2, 128).transpose(2, 0, 1)).astype(np.float32)

    core_ids = list(range(NCORES))
    in_maps = []
    for i in core_ids:
        xc = np.ascontiguousarray(
            xtc_full[:, :, :, i * B_LOC:(i + 1) * B_LOC])
        in_maps.append({
            "xtc": xc,
            "w1t": w1t, "w2t": w2t, "w3z": w3z, "b1s": b1s, "b2s": b2s,
        })

    import os
    trace = bool(os.environ.get("BASS_TRACE"))
    res = run_bass_kernel_spmd(nc, in_maps, core_ids, trace=trace)
    last_results = res

    out_full = np.empty((B, C), dtype=np.float32)
    for i in core_ids:
        o = res.results[i]["out"].reshape(4, 32, SEC)[:, :20]
        o = o.reshape(4, NSEC, 5, SEC)
        # o[j, h, r, b] = out[class 4r+j, batch 512h+b]
        o = o.transpose(1, 3, 2, 0).reshape(B_LOC, 20)[:, :C]
        out_full[i * B_LOC:(i + 1) * B_LOC] = o
    out_full += b3[None, :]
    return out_full


# revision 18
# speedup vs baseline: 1.2057x; 1.0114x over previous
"""Trainium2 Bass kernel for nn_CWDiscriminator (per-class 3-layer MLP).

reference:
    x = inputs.transpose(0, 2, 1)            # (B, C, F)
    h = relu(einsum('bcf,cfg->bcg', x, W1) + b1)
    h = relu(einsum('bcf,cfg->bcg', h, W2) + b2)
    out = einsum('bcf,cf->bc', h, W3) + b3   # (B, C)

B=16384, F=256, C=19. Data-parallel over B across 8 NeuronCores
(B_loc = 2048 per core). CLASS-MAJOR schedule: outer loop over classes,
inner over four 512-column batch sections, so HBM demand is uniform
(~1.3 MB per 8.6 us of compute) instead of front-loaded; X prefetches
several classes ahead on the sync ring while weights stream
class-by-class on the scalar ring.

Per class c, per section h:
  - GEMM1 (bf16): H1.T = W1[c].T @ X.T -> PSUM, ACT evicts fused
    bias+ReLU to bf16.
  - GEMM2 (bf16): H2.T = W2[c].T @ H1.T -> PSUM, DVE evicts fused
    bias+ReLU to bf16.
  - GEMM3: col-tiled, batched per group of 4 classes (r = c//4,
    strip j = c%4). The four classes' matmuls go to the four
    32-partition column groups via tile_position=(0,32j) and run
    concurrently, all accumulating into ONE shared PSUM bank at
    partition 32j + 5h + r (5h+r is a bijection onto 0..19, so each
    strip's 20 rows are dense); section h is selected by leading-zero
    columns in the lhsT. A zero matmul at kernel start sets
    has_written for the bank (doubling as PE warm-up), so strip
    matmuls never need start=True (whose bank-wide clear would race
    between concurrent strips).
Output per core: one padded DMA to dram [4,32,SEC-cols-per-...]; host
takes rows :20 per strip, reshapes, transposes, adds b3.
"""

import sys
import types

import numpy as np
import ml_dtypes

B, F, C = 16384, 256, 19
NCORES = 8
B_LOC = B // NCORES          # 2048
NSEC = 4
SEC = B_LOC // NSEC          # 512
BF16 = ml_dtypes.bfloat16

XT_BUFS = 8                  # X prefetch depth in classes (1 MB each)
# strip j = c % 4, row r = c // 4; classes per strip column
STRIP_N = [5, 5, 5, 4]


# ---------------------------------------------------------------------------
# axon environment shims (NTFF profile hook + artifact upload stub) and the
# one-wait-per-instruction legalizer this walrus build requires.
# ---------------------------------------------------------------------------

def _setup_axon_env():
    if 'antenv.axon_hooks' not in sys.modules:
        mod = types.ModuleType('antenv.axon_hooks')
        mod._hook = None
        mod.set_axon_ntff_profile_hook = lambda h: setattr(mod, '_hook', h)
        mod.get_axon_ntff_profile_hook = lambda: mod._hook
        sys.modules['antenv.axon_hooks'] = mod
        try:
            import antenv
            antenv.axon_hooks = mod
        except ImportError:
            pass
        try:
            from trn_agent_boot.trn_boot import _ntff_profile_via_ctypes
            mod._hook = _ntff_profile_via_ctypes('/opt/axon/libaxon_pjrt.so')
        except Exception:
            pass
    import concourse.bass_utils as bu
    bu.upload_artifacts = lambda tmpdir: 'file://' + str(tmpdir)


def _legalize_waits(nc):
    """walrus accepts at most ONE sync wait per engine instruction (2 for
    EventSemaphore). Split extras onto preceding same-engine NoOps."""
    import concourse.mybir as mybir
    n_split = 0
    for fn in nc.m.functions:
        for bb in fn.blocks:
            insts = bb.instructions
            out = []
            for inst in insts:
                si = inst.sync_info
                ow = list(si.on_wait) if si is not None and si.on_wait else []
                cap = 2 if inst.opcode == "EventSemaphore" else 1
                if len(ow) > cap:
                    keep = ow[-cap:]
                    for k, w in enumerate(ow[:-cap]):
                        nop = mybir.InstNoOp(
                            name=f"{inst.name}-wsplit{k}",
                            engine=inst.engine,
                            ins=[],
                            outs=[],
                            sync_info=mybir.SyncInfo(on_wait=[w], on_update=[]),
                        )
                        out.append(nop)
                        n_split += 1
                    inst.sync_info = mybir.SyncInfo(
                        on_wait=keep,
                        on_update=list(si.on_update) if si.on_update else [],
                    )
                out.append(inst)
            insts[:] = out
    return n_split


# ---------------------------------------------------------------------------
# device program
# ---------------------------------------------------------------------------

_CACHE = {}
last_results = None  # BassKernelResults of the most recent run (for test.py)

# class-range chunks for weight loads (finer first so class 0 starts early)
W_CHUNKS = [(0, 1), (1, 2), (2, 4), (4, 7), (7, 11), (11, 15), (15, C)]


def _build_program():
    from contextlib import ExitStack
    import concourse.bass as bass
    import concourse.mybir as mybir
    import concourse.tile as tile

    F32 = mybir.dt.float32
    B16 = mybir.dt.bfloat16

    nc = bass.Bass()

    # xtc[c, p, k, b] = x[b, 128k+p, c]  (class-major, contiguous per class)
    xtc = nc.declare_dram_parameter("xtc", [C, 128, 2, B_LOC], B16,
                                    isOutput=False)
    w1t = nc.declare_dram_parameter("w1t", [128, C, 2, 2, 128], B16,
                                    isOutput=False)
    w2t = nc.declare_dram_parameter("w2t", [128, C, 2, 2, 128], B16,
                                    isOutput=False)
    # w3z[p, c, k, h, i] = (i == 8h + c//4) * W3[c, 128k+p]
    w3z = nc.declare_dram_parameter("w3z", [128, C, 2, NSEC, 20], B16,
                                    isOutput=False)
    b1s = nc.declare_dram_parameter("b1s", [128, C, 2], F32, isOutput=False)
    b2s = nc.declare_dram_parameter("b2s", [128, C, 2], F32, isOutput=False)
    # out[j, q, b]: q = 5h+r -> class 4r+j, batch column 512h+b
    # (rows 20..31 per strip are padding so one 128-partition DMA works)
    out = nc.declare_dram_parameter("out", [4, 32, SEC], F32,
                                    isOutput=True)

    with ExitStack() as ctx:
        tc = ctx.enter_context(tile.TileContext(nc))

        consts = ctx.enter_context(tc.tile_pool(name="consts", bufs=1))
        xt_pool = ctx.enter_context(tc.tile_pool(name="xt", bufs=XT_BUFS))
        h1_pool = ctx.enter_context(tc.tile_pool(name="h1p", bufs=8))
        h2_pool = ctx.enter_context(tc.tile_pool(name="h2p", bufs=24))
        out_pool = ctx.enter_context(tc.tile_pool(name="outp", bufs=1))

        ps_g = ctx.enter_context(
            tc.tile_pool(name="ps_g", bufs=7, space="PSUM"))
        ps_3 = ctx.enter_context(
            tc.tile_pool(name="ps_3", bufs=1, space="PSUM"))

        # ---- SBUF tiles for weights/biases
        w1sb = consts.tile([128, C, 2, 2, 128], B16)
        w2sb = consts.tile([128, C, 2, 2, 128], B16)
        w3sb = consts.tile([128, C, 2, NSEC, 20], B16)
        b1sb = consts.tile([128, C, 2], F32)
        b2sb = consts.tile([128, C, 2], F32)

        # ---- DMA: X classes on the sync ring (pool-paced prefetch),
        # weights on the scalar ring in consumption order.
        xts = [xt_pool.tile([128, 2, B_LOC], B16, tag="xt",
                            name=f"xt{c}") for c in range(C)]
        nc.sync.dma_start(b1sb[:], b1s[:])
        nc.sync.dma_start(b2sb[:], b2s[:])
        nc.sync.dma_start(xts[0][:, :, 0:SEC], xtc[0, :, :, 0:SEC])
        nc.sync.dma_start(w1sb[:, 0:1], w1t[:, 0:1])
        nc.sync.dma_start(xts[0][:, :, SEC:2 * SEC],
                          xtc[0, :, :, SEC:2 * SEC])
        nc.sync.dma_start(xts[0][:, :, 2 * SEC:],
                          xtc[0, :, :, 2 * SEC:])
        nc.sync.dma_start(w2sb[:, 0:1], w2t[:, 0:1])
        nc.sync.dma_start(xts[1][:, :, 0:B_LOC // 2],
                          xtc[1, :, :, 0:B_LOC // 2])
        nc.sync.dma_start(xts[1][:, :, B_LOC // 2:],
                          xtc[1, :, :, B_LOC // 2:])
        for n, (c0, c1) in enumerate(W_CHUNKS[1:]):
            nc.sync.dma_start(w1sb[:, c0:c1], w1t[:, c0:c1])
            nc.sync.dma_start(w2sb[:, c0:c1], w2t[:, c0:c1])
            if n == 2:
                nc.sync.dma_start(w3sb[:], w3z[:])
            if n + 2 < C:
                nc.sync.dma_start(xts[n + 2][:], xtc[n + 2])
        for c in range(len(W_CHUNKS) + 1, C):
            nc.sync.dma_start(xts[c][:], xtc[c])

        # ---- zero matmuls: set has_written across all 4 GEMM3 banks and
        # warm the PE while the first DMA chunks land.
        wu_l = consts.tile([128, 128], B16)
        wu_r = consts.tile([128, 512], B16)
        nc.vector.memset(wu_l[:], 0.0)
        nc.vector.memset(wu_r[:], 0.0)
        ps3 = ps_3.tile([128, SEC], mybir.dt.float32, tag="ps3")
        nc.tensor.matmul(ps3[:], wu_l[:], wu_r[:, 0:SEC],
                         start=True, stop=False, skip_group_check=True)
        wu_ps = ps_g.tile([128, 512], mybir.dt.float32, tag="pg")
        for i in range(6):
            nc.tensor.matmul(wu_ps[:], wu_l[:], wu_r[:],
                             start=True, stop=True)

        w1v = w1sb[:]
        w2v = w2sb[:]
        w3v = w3sb[:]

        h1_t = [None, None]      # per class parity: list of 4 section tiles
        h2_t = [None] * C        # h2 tiles for the current group of 4

        def burst(r):
            # GEMM3 for classes 4r..4r+3, all sections; strips concurrent.
            # Section h lands at partition offset 5h + r within strip j via
            # leading-zero columns in the lhsT (width 5h+r+1 <= 20); 5h+r
            # is a bijection onto 0..19 so the 20 partitions per strip are
            # dense and the out-DMA is contiguous.
            for h in range(NSEC):
                for k in range(2):
                    for j in range(4):
                        c = 4 * r + j
                        if c >= C:
                            continue
                        w = 5 * h + r + 1
                        last = (r == 4 and h == NSEC - 1 and k == 1
                                and j == 2)
                        nc.tensor.matmul(
                            ps3[32 * j:32 * j + w, :],
                            w3v[:, c, k, h, 0:w],
                            h2_t[c][h][:, k, :],
                            start=False, stop=last,
                            tile_position=(0, 32 * j),
                            skip_group_check=True)

        for cc in range(C + 1):
            if cc >= 5 and (cc - 5) % 4 == 0 and cc < C:
                burst((cc - 5) // 4)
            if cc < C:
                c = cc
                h1s = []
                for h in range(NSEC):
                    h1 = h1_pool.tile([128, 2, SEC], B16, tag="h1")
                    h1s.append(h1)
                    for m in range(2):
                        pg = ps_g.tile([128, SEC], mybir.dt.float32,
                                       tag="pg")
                        for k in range(2):
                            nc.tensor.matmul(
                                pg[:], w1v[:, c, k, m, :],
                                xts[c][:, k, h * SEC:(h + 1) * SEC],
                                start=(k == 0), stop=(k == 1))
                        nc.scalar.activation(
                            h1[:, m, :], pg[:],
                            mybir.ActivationFunctionType.Relu,
                            bias=b1sb[:, c, m:m+1])
                h1_t[c % 2] = h1s
            if cc >= 1:
                c = cc - 1
                h1s = h1_t[c % 2]
                h2s = []
                for h in range(NSEC):
                    h2 = h2_pool.tile([128, 2, SEC], B16, tag="h2")
                    h2s.append(h2)
                    for m in range(2):
                        pg = ps_g.tile([128, SEC], mybir.dt.float32,
                                       tag="pg")
                        for k in range(2):
                            nc.tensor.matmul(
                                pg[:], w2v[:, c, k, m, :],
                                h1s[h][:, k, :],
                                start=(k == 0), stop=(k == 1))
                        if cc == C:
                            # last class: ACT is idle (no GEMM1 left) and
                            # evicts with no queue lag, so the final
                            # GEMM3 burst's h2 deps resolve immediately.
                            nc.scalar.activation(
                                h2[:, m, :], pg[:],
                                mybir.ActivationFunctionType.Relu,
                                bias=b2sb[:, c, m:m+1])
                        else:
                            nc.vector.tensor_scalar(
                                h2[:, m, :], pg[:],
                                b2sb[:, c, m:m+1], 0.0,
                                mybir.AluOpType.add, mybir.AluOpType.max)
                h2_t[c] = h2s
            if cc == C:
                burst(4)

        # ---- eviction: PSUM->SBUF copy split over DVE/ACT, then one
        # 128-partition DMA (pad rows land in the dram padding).
        out_sb = out_pool.tile([128, SEC], F32, tag="osb")
        nc.vector.tensor_copy(out_sb[:, 0:SEC // 2], ps3[:, 0:SEC // 2])
        nc.scalar.activation(out_sb[:, SEC // 2:], ps3[:, SEC // 2:],
                             mybir.ActivationFunctionType.Copy)
        nc.sync.dma_start(out[:].rearrange("j q b -> (j q) b"), out_sb[:])

    _legalize_waits(nc)
    return nc


def _get_program():
    if 'nc' not in _CACHE:
        _setup_axon_env()
        _CACHE['nc'] = _build_program()
    return _CACHE['nc']


# ---------------------------------------------------------------------------
# host wrapper
# ---------------------------------------------------------------------------

def kernel(inputs, W1, b1, W2, b2, W3, b3):
    global last_results
    from concourse.bass_utils import run_bass_kernel_spmd

    nc = _get_program()

    inputs = np.asarray(inputs)
    W1 = np.asarray(W1, dtype=np.float32)
    b1 = np.asarray(b1, dtype=np.float32)
    W2 = np.asarray(W2, dtype=np.float32)
    b2 = np.asarray(b2, dtype=np.float32)
    W3 = np.asarray(W3, dtype=np.float32)
    b3 = np.asarray(b3, dtype=np.float32)

    # host-side layout prep: xtc[c, p, k, b] = x[b, 128k+p, c]
    xbf = inputs.reshape(B, 2, 128, C).astype(BF16)
    xtc_full = xbf.transpose(3, 2, 1, 0)      # (C, 128, 2, B)

    # lhsT tiles: w{1,2}t[p, c, k, m, j] = W[c, 128k+p, 128m+j]
    w1t = np.ascontiguousarray(
        W1.reshape(C, 2, 128, 2, 128).transpose(2, 0, 1, 3, 4)).astype(BF16)
    w2t = np.ascontiguousarray(
        W2.reshape(C, 2, 128, 2, 128).transpose(2, 0, 1, 3, 4)).astype(BF16)
    # w3z[p, c, k, h, i] = (i == 5h + c//4) * W3[c, 128k+p]
    w3z = np.zeros((128, C, 2, NSEC, 20), dtype=np.float32)
    for c in range(C):
        for h in range(NSEC):
            w3z[:, c, 0, h, 5 * h + c // 4] = W3[c, :128]
            w3z[:, c, 1, h, 5 * h + c // 4] = W3[c, 128:]
    w3z = w3z.astype(BF16)
    # b1s[p, c, m] = b1[c, 128m+p]
    b1s = np.ascontiguousarray(
        b1.reshape(C, 2, 128).transpose(2, 0, 1)).astype(np.float32)
    b2s = np.ascontiguousarray(
        b2.reshape(C, 2, 128).transpose(2, 0, 1)).astype(np.float32)

    core_ids = list(range(NCORES))
    in_maps = []
    for i in core_ids:
        xc = np.ascontiguousarray(
            xtc_full[:, :, :, i * B_LOC:(i + 1) * B_LOC])
        in_maps.append({
            "xtc": xc,
            "w1t": w1t, "w2t": w2t, "w3z": w3z, "b1s": b1s, "b2s": b2s,
        })

    import os
    trace = bool(os.environ.get("BASS_TRACE"))
    res = run_bass_kernel_spmd(nc, in_maps, core_ids, trace=trace)
    last_results = res

    out_full = np.empty((B, C), dtype=np.float32)
    for i in core_ids:
        o = res.results[i]["out"].reshape(4, 32, SEC)[:, :20]
        o = o.reshape(4, NSEC, 5, SEC)
        # o[j, h, r, b] = out[class 4r+j, batch 512h+b]
        o = o.transpose(1, 3, 2, 0).reshape(B_LOC, 20)[:, :C]
        out_full[i * B_LOC:(i + 1) * B_LOC] = o
    out_full += b3[None, :]
    return out_full
